# revision 21
# baseline (speedup 1.0000x reference)
"""Fused transformer block (attention + MLP) on 8 trn2 NeuronCores.

Sharding: tensor-parallel over heads across ALL 8 cores — core c owns heads
{2c, 2c+1} for BOTH batches. Attention runs head-within-pair (hh) major;
each core's pre-projection attention outputs are exchanged with TWO 8-core
AllToAlls (one per hh, 512KB each) so the first fully overlaps the second
half's compute. After the exchange core r holds all 16 heads for its
(batch, T-shard) = (r//4, r%4) slice of 512 tokens and runs projection +
full FFN locally (sequence parallel). Output shards are gathered on host.

On-chip layout is "T-last" (feature dim on partitions, tokens on the free
axis) so no transposes are ever needed. V is built in natural [T, hs]
layout with a ones-column appended so the softmax denominator falls out of
the same matmul. Logits are tiny (|s| < ~1), so softmax needs no max
subtraction.
"""

import sys

for _p in ("/opt/trn_rl_repo",):
    if _p not in sys.path:
        sys.path.append(_p)

import numpy as np
import ml_dtypes

import concourse.bass as bass
import concourse.tile as tile
from concourse import bacc, mybir
from concourse.bass_utils import run_bass_kernel_spmd

BF16 = mybir.dt.bfloat16
FP8 = mybir.dt.float8e4
F32 = mybir.dt.float32
DR = mybir.MatmulPerfMode.DoubleRow
AF = mybir.ActivationFunctionType
ALU = mybir.AluOpType

N_CORES = 8
B, T, C = 2, 2048, 1024
H, HS = 16, 64
F = 4 * C
TS = T // 4
CC = C // 128
FB = F // 128
SCALE = float(C) ** -0.5


def build_program(nc: bass.Bass):
    xt_bf = nc.dram_tensor("xt_bf", [B, 4, 128, 2, T], FP8, kind="ExternalInput").ap()
    xts_f = nc.dram_tensor("xts_f", [CC, 128, TS], F32, kind="ExternalInput").ap()
    wqkv_d = nc.dram_tensor("wqkv", [4, 128, 2, 384], FP8, kind="ExternalInput").ap()
    wp_d = nc.dram_tensor("wp", [2, 4, 128, C], BF16, kind="ExternalInput").ap()
    w1_d = nc.dram_tensor("w1", [CC, 128, F], BF16, kind="ExternalInput").ap()
    w2_d = nc.dram_tensor("w2", [FB, 128, C], BF16, kind="ExternalInput").ap()
    bp_d = nc.dram_tensor("bp", [128, CC], F32, kind="ExternalInput").ap()
    b1_d = nc.dram_tensor("b1", [128, FB], F32, kind="ExternalInput").ap()
    b2_d = nc.dram_tensor("b2", [128, CC], F32, kind="ExternalInput").ap()
    out_d = nc.dram_tensor("outT", [CC, 128, TS], F32, kind="ExternalOutput").ap()

    with tile.TileContext(nc) as tc:
        _emit(nc, tc, xt_bf, xts_f, wqkv_d, wp_d, w1_d, w2_d,
              bp_d, b1_d, b2_d, out_d)


def _emit(nc, tc, xt_bf, xts_f, wqkv_d, wp_d, w1_d, w2_d,
          bp_d, b1_d, b2_d, out_d):
    from contextlib import ExitStack

    ctx = ExitStack()
    with ctx:
        st = ctx.enter_context(tc.tile_pool(name="static", bufs=1))
        w2p = ctx.enter_context(tc.tile_pool(name="w2s", bufs=3))
        outp = ctx.enter_context(tc.tile_pool(name="outp", bufs=4))
        rcp = ctx.enter_context(tc.tile_pool(name="rcp", bufs=2))
        dram0 = ctx.enter_context(tc.tile_pool(name="dram0", bufs=1, space="DRAM"))
        dram1 = ctx.enter_context(tc.tile_pool(name="dram1", bufs=1, space="DRAM"))

        # attention-phase SBUF (freed for w1-late/hT/x1 afterwards)
        attn_sb_ctx = tc.tile_pool(name="attnsb", bufs=1)
        asb = attn_sb_ctx.__enter__()
        expp_ctx = tc.tile_pool(name="expp", bufs=8)
        expp = expp_ctx.__enter__()

        psc_ctx = tc.tile_pool(name="psc", bufs=3, space="PSUM")
        psc = psc_ctx.__enter__()
        ps_ctx = tc.tile_pool(name="ps", bufs=2, space="PSUM")
        ps = ps_ctx.__enter__()

        # ---- weight loads first on gpsimd (QKV matmuls gate on these)
        wqkv_sb = []
        for cp in range(4):
            t_ = asb.tile([128, 2, 384], FP8, tag=f"wqkv{cp}", name=f"wqkv_sb{cp}")
            nc.gpsimd.dma_start(t_[:], wqkv_d[cp])
            wqkv_sb.append(t_)
        wk_sb = [t[:, :, 128:256] for t in wqkv_sb]
        wq_sb = [t[:, :, 0:128] for t in wqkv_sb]
        wv_sb = [t[:, :, 256:384] for t in wqkv_sb]
        bp_all = st.tile([128, CC], F32, tag="bp", name="bp_all")
        nc.gpsimd.dma_start(bp_all[:], bp_d)
        b1_all = st.tile([128, FB], F32, tag="b1", name="b1_all")
        nc.gpsimd.dma_start(b1_all[:], b1_d)
        b2_all = st.tile([128, CC], F32, tag="b2", name="b2_all")
        nc.gpsimd.dma_start(b2_all[:], b2_d)

        # ---- x loads: batch 0 chunk-major first, batch 1 after
        xt_sb = [[asb.tile([128, 2, T], FP8, tag=f"xt{b}{cp}", name=f"xt_sb{b}{cp}")
                  for cp in range(4)] for b in range(B)]
        for b in range(B):
            for q4 in range(4):
                for cp in range(4):
                    nc.sync.dma_start(xt_sb[b][cp][:, :, q4 * 512:(q4 + 1) * 512],
                                      xt_bf[b, cp][:, :, q4 * 512:(q4 + 1) * 512])
        wp_sb = [[None] * 4 for _ in range(2)]
        for hh in range(2):
            for t4 in range(4):
                t_ = st.tile([128, C], BF16, tag=f"wp{hh}_{t4}",
                             name=f"wp_sb{hh}_{t4}")
                nc.sync.dma_start(t_[:], wp_d[hh, t4])
                wp_sb[hh][t4] = t_
        xs_sb = []
        for cc in range(CC):
            xs = st.tile([128, TS], F32, tag=f"xs{cc}", name=f"xs{cc}")
            nc.sync.dma_start(xs[:], xts_f[cc])
            xs_sb.append(xs)
        mask_big = st.tile([128, 896], BF16, tag="mask", name="mask_big")
        nc.gpsimd.memset(mask_big[:], 1.0)
        nc.gpsimd.affine_select(mask_big[:], mask_big[:], pattern=[[1, 896]],
                                compare_op=ALU.is_ge, fill=0.0, base=-384,
                                channel_multiplier=-1)

        qT = [asb.tile([128, T], BF16, tag=f"qT{b}", name=f"qT_sb{b}")
              for b in range(B)]
        kT = [asb.tile([128, T], BF16, tag=f"kT{b}", name=f"kT_sb{b}")
              for b in range(B)]
        attnT = [asb.tile([128, T], FP8, tag=f"attnT{b}", name=f"attnT_sb{b}")
                 for b in range(B)]
        v_sb = [[None] * (T // 128) for _ in range(B)]
        at_sb = [[None] * 4 for _ in range(2)]
        exq = {}  # (b, qt, hh) -> list of exp tiles

        a2a_in = [dram0.tile([512, TS], FP8, name="a2a_in0"),
                  dram1.tile([512, TS], FP8, name="a2a_in1")]
        a2a_out = [dram0.tile([512, TS], FP8, name="a2a_out0"),
                   dram1.tile([512, TS], FP8, name="a2a_out1")]

        # w1 cc 0-3 first halves: loaded early (during attention)
        w1_sb = [[None, None] for _ in range(CC)]
        for cc in range(4):
            w1_sb[cc][0] = st.tile([128, 2048], BF16, tag=f"w1e{cc}",
                                   name=f"w1_sb{cc}_0")

        def qk(b, half):
            for nm, w_sb, dst in (("k", wk_sb, kT), ("q", wq_sb, qT)):
                acc = psc.tile([128, 1024], F32, tag="psc",
                               name=f"ps_{nm}{b}{half}")
                for t2 in range(2):
                    tt = 2 * half + t2
                    for cp in range(4):
                        nc.tensor.matmul(
                            acc[:, t2 * 512:(t2 + 1) * 512],
                            w_sb[cp][:],
                            xt_sb[b][cp][:, :, tt * 512:(tt + 1) * 512],
                            start=(cp == 0), stop=(cp == 3),
                            perf_mode=DR,
                        )
                nc.vector.tensor_copy(
                    dst[b][:, half * 1024:(half + 1) * 1024], acc[:])

        def v_tiles(b, tk0, tk1):
            for tk in range(tk0, tk1):
                vt = asb.tile([128, 2 * 65], BF16, tag=f"v{b}{tk}",
                              name=f"v_sb{b}{tk}")
                nc.gpsimd.memset(vt[:], 32.0)
                acc = ps.tile([128, 128], F32, tag="ps", name=f"ps_v{b}{tk}")
                for cp in range(4):
                    nc.tensor.matmul(
                        acc[:],
                        xt_sb[b][cp][:, :, tk * 128:(tk + 1) * 128],
                        wv_sb[cp][:],
                        start=(cp == 0), stop=(cp == 3),
                        perf_mode=DR,
                    )
                src = acc.rearrange("p (h d) -> p h d", h=2)
                dstv = vt.rearrange("p (h d) -> p h d", h=2, d=65)[:, :, 0:64]
                nc.vector.tensor_copy(dstv, src)
                v_sb[b][tk] = vt

        def sc(b, qt, hh):
            nkc = 4 * (qt + 1)
            p0 = 64 * hh
            for b2i in range(nkc // 2):
                sct = psc.tile([128, 1024], F32, tag="psc",
                               name=f"psc{b}{hh}{qt}_{b2i}")
                for j in range(2):
                    kc = 2 * b2i + j
                    nc.tensor.matmul(
                        sct[:, j * 512:(j + 1) * 512],
                        kT[b][p0:p0 + 64, kc * 128:(kc + 1) * 128],
                        qT[b][p0:p0 + 64, qt * 512:(qt + 1) * 512],
                        start=True, stop=True,
                    )
                ext = expp.tile([128, 1024], BF16, tag="expp",
                                name=f"ex{b}{hh}{qt}_{b2i}")
                nc.scalar.activation(ext[:], sct[:], AF.Exp, scale=SCALE / 1024.0)
                for j in range(2):
                    kc = 2 * b2i + j
                    if kc >= 4 * qt:
                        dd = kc * 128 - qt * 512
                        nc.vector.tensor_mul(
                            ext[:, j * 512:(j + 1) * 512],
                            ext[:, j * 512:(j + 1) * 512],
                            mask_big[:, 384 - dd:896 - dd],
                        )
                exq.setdefault((b, qt, hh), []).append(ext)

        def wv(b, qt, hh):
            nkc = 4 * (qt + 1)
            p0 = 64 * hh
            wv_acc = ps.tile([65, 512], F32, tag="ps", name=f"pwv{b}{hh}{qt}")
            for b2i in range(nkc // 2):
                ext = exq[(b, qt, hh)][b2i]
                for j in range(2):
                    kc = 2 * b2i + j
                    nc.tensor.matmul(
                        wv_acc[:],
                        v_sb[b][kc][:, hh * 65:hh * 65 + 65],
                        ext[:, j * 512:(j + 1) * 512],
                        start=(kc == 0), stop=(kc == nkc - 1),
                    )
            den = rcp.tile([1, 512], F32, tag="den", name=f"den{b}{hh}{qt}")
            nc.vector.tensor_copy(den[:], wv_acc[64:65, :])
            rc = rcp.tile([1, 512], F32, tag="rc", name=f"rc{b}{hh}{qt}")
            nc.vector.reciprocal_approx_fast(rc[:], den[:])
            rb = rcp.tile([64, 512], F32, tag="rb", name=f"rb{b}{hh}{qt}")
            nc.gpsimd.partition_broadcast(rb[:], rc[:])
            nc.vector.tensor_mul(
                attnT[b][p0:p0 + 64, qt * 512:(qt + 1) * 512],
                wv_acc[0:64, :], rb[:],
            )
            # ship this (b, q-tile, hh) slice to destination core 4b+qt
            j = 4 * b + qt
            nc.sync.dma_start(a2a_in[hh][j * 64:(j + 1) * 64, :],
                              attnT[b][p0:p0 + 64, qt * 512:(qt + 1) * 512])

        # ---- attention, hh-major ----
        qk(0, 0)
        v_tiles(0, 0, 4)
        sc(0, 0, 0)
        wv(0, 0, 0)
        qk(0, 1)
        v_tiles(0, 4, 8)
        sc(0, 1, 0)
        wv(0, 1, 0)
        qk(1, 0)
        v_tiles(0, 8, 12)
        sc(0, 2, 0)
        wv(0, 2, 0)
        qk(1, 1)
        v_tiles(0, 12, 16)
        sc(0, 3, 0)
        wv(0, 3, 0)
        v_tiles(1, 0, 4)
        v_tiles(1, 4, 8)
        sc(1, 0, 0)
        wv(1, 0, 0)
        v_tiles(1, 8, 12)
        sc(1, 1, 0)
        wv(1, 1, 0)
        v_tiles(1, 12, 16)
        # w1 cc0-3 first-half loads: transfers overlap late attention
        for cc in range(4):
            nc.gpsimd.dma_start(w1_sb[cc][0][:], w1_d[cc][:, 0:2048])
        sc(1, 2, 0)
        wv(1, 2, 0)
        sc(1, 3, 0)
        wv(1, 3, 0)

        nc.gpsimd.collective_compute(
            "AllToAll", ALU.bypass,
            replica_groups=[[0, 1, 2, 3, 4, 5, 6, 7]],
            ins=[a2a_in[0].opt()], outs=[a2a_out[0].opt()],
        )
        # emitted before collective #2 so they only gate on #1; the hh=1
        # a2a stores queue behind them, which is harmless (needed late)
        for t4 in range(4):
            t8 = st.tile([128, TS], FP8, tag=f"at0q{t4}", name=f"at0q{t4}")
            nc.sync.dma_start(t8[:], a2a_out[0][t4 * 128:(t4 + 1) * 128, :])
            t_ = st.tile([128, TS], BF16, tag=f"at0_{t4}", name=f"at0_{t4}")
            nc.vector.tensor_copy(t_[:], t8[:])
            at_sb[0][t4] = t_

        for b in range(B):
            for qt in range(4):
                sc(b, qt, 1)
                wv(b, qt, 1)

        nc.gpsimd.collective_compute(
            "AllToAll", ALU.bypass,
            replica_groups=[[0, 1, 2, 3, 4, 5, 6, 7]],
            ins=[a2a_in[1].opt()], outs=[a2a_out[1].opt()],
        )

        # attention pools done; free SBUF + PSUM for proj/FFN
        ps_ctx.__exit__(None, None, None)
        psc_ctx.__exit__(None, None, None)
        expp_ctx.__exit__(None, None, None)
        attn_sb_ctx.__exit__(None, None, None)

        late_ctx = tc.tile_pool(name="late", bufs=1)
        late = late_ctx.__enter__()
        for cc in range(4, CC):
            w1_sb[cc][0] = late.tile([128, 2048], BF16, tag=f"w1l{cc}",
                                     name=f"w1_sb{cc}_0")
            nc.sync.dma_start(w1_sb[cc][0][:], w1_d[cc][:, 0:2048])

        # these wait on AllToAll #2; only later w1/w2 loads queue behind them
        for t4 in range(4):
            t8 = st.tile([128, TS], FP8, tag=f"at1q{t4}", name=f"at1q{t4}")
            nc.sync.dma_start(t8[:], a2a_out[1][t4 * 128:(t4 + 1) * 128, :])
            t_ = st.tile([128, TS], BF16, tag=f"at1_{t4}", name=f"at1_{t4}")
            nc.vector.tensor_copy(t_[:], t8[:])
            at_sb[1][t4] = t_

        # ---- proj on the destination core: full Wproj @ attn T-shard ----
        # hh=0 half overlaps the second AllToAll.
        projps_ctx = tc.tile_pool(name="projps", bufs=8, space="PSUM")
        projps = projps_ctx.__enter__()
        pacc = [projps.tile([128, TS], F32, tag=f"pp{cb}", bufs=1,
                            name=f"ps_pj{cb}") for cb in range(CC)]
        x1f = [None] * CC
        x1b = [None] * CC
        for hh in range(2):
            for t4 in range(4):
                for cb in range(CC):
                    nc.tensor.matmul(
                        pacc[cb][:], wp_sb[hh][t4][:, cb * 128:(cb + 1) * 128],
                        at_sb[hh][t4][:],
                        start=(hh == 0 and t4 == 0), stop=(hh == 1 and t4 == 3))
                    if hh == 1 and t4 == 3:
                        xf = late.tile([128, TS], F32, tag=f"x1f{cb}",
                                       name=f"x1f{cb}")
                        nc.vector.scalar_tensor_tensor(
                            xf[:], pacc[cb][:], bp_all[:, cb:cb+1], xs_sb[cb][:],
                            ALU.add, ALU.add)
                        x1f[cb] = xf
                        xb = late.tile([128, TS], BF16, tag=f"x1b{cb}",
                                       name=f"x1b{cb}")
                        nc.vector.tensor_copy(xb[:], xf[:])
                        x1b[cb] = xb
        projps_ctx.__exit__(None, None, None)

        for cc in range(CC):
            w1_sb[cc][1] = late.tile([128, 2048], BF16, tag=f"w1m{cc}",
                                     name=f"w1_sb{cc}_1")
            nc.sync.dma_start(w1_sb[cc][1][:], w1_d[cc][:, 2048:4096])


        # ---- FFN1 with FFN2 group A (cb 0-3) interleaved ----
        fps_ctx = tc.tile_pool(name="fps", bufs=2, space="PSUM")
        fps = fps_ctx.__enter__()
        pf2a_ctx = tc.tile_pool(name="pf2a", bufs=6, space="PSUM")
        pf2a = pf2a_ctx.__enter__()
        accA = [pf2a.tile([128, TS], F32, tag=f"pfa{cb}", bufs=1,
                          name=f"ps_oa{cb}") for cb in range(6)]
        hT = [late.tile([128, TS], BF16, tag=f"hT{fb}", name=f"hT{fb}")
              for fb in range(FB)]
        for fb in range(FB):
            w1h, fo = fb // 16, fb % 16
            acc = fps.tile([128, TS], F32, tag="fps", name=f"ps_h{fb}")
            for cc in range(CC):
                nc.tensor.matmul(
                    acc[:],
                    w1_sb[cc][w1h][:, fo * 128:(fo + 1) * 128],
                    x1b[cc][:],
                    start=(cc == 0), stop=(cc == CC - 1))
            nc.scalar.activation(hT[fb][:], acc[:], AF.Relu, bias=b1_all[:, fb:fb+1])
            wt = w2p.tile([128, 768], BF16, tag="w2s", name=f"w2ta{fb}")
            nc.sync.dma_start(wt[:], w2_d[fb][:, 0:768])
            for cb in range(6):
                nc.tensor.matmul(
                    accA[cb][:], wt[:, cb * 128:(cb + 1) * 128], hT[fb][:],
                    start=(fb == 0), stop=(fb == FB - 1))
        for cb in range(6):
            ot = outp.tile([128, TS], F32, tag="outp", name=f"ot{cb}")
            nc.vector.scalar_tensor_tensor(ot[:], accA[cb][:], b2_all[:, cb:cb+1],
                                           x1f[cb][:], ALU.add, ALU.add)
            nc.sync.dma_start(out_d[cb], ot[:])
        pf2a_ctx.__exit__(None, None, None)

        # ---- FFN2 group B ----
        pf2_ctx = tc.tile_pool(name="pf2", bufs=2, space="PSUM")
        pf2 = pf2_ctx.__enter__()
        accB = [pf2.tile([128, TS], F32, tag=f"pfb{cb}", bufs=1,
                         name=f"ps_ob{cb}") for cb in range(2)]
        for fc in range(FB):
            wt = w2p.tile([128, 256], BF16, tag="w2s", name=f"w2tb{fc}")
            nc.sync.dma_start(wt[:], w2_d[fc][:, 768:1024])
            for cb in range(2):
                nc.tensor.matmul(
                    accB[cb][:], wt[:, cb * 128:(cb + 1) * 128], hT[fc][:],
                    start=(fc == 0), stop=(fc == FB - 1))
        for cb2 in range(2):
            cb = cb2 + 6
            ot = outp.tile([128, TS], F32, tag="outp", name=f"ot{cb}")
            nc.vector.scalar_tensor_tensor(ot[:], accB[cb2][:], b2_all[:, cb:cb+1],
                                           x1f[cb][:], ALU.add, ALU.add)
            nc.sync.dma_start(out_d[cb], ot[:])
        pf2_ctx.__exit__(None, None, None)
        fps_ctx.__exit__(None, None, None)
        late_ctx.__exit__(None, None, None)


_CACHED = None


def _get_compiled():
    global _CACHED
    if _CACHED is None:
        nc = bacc.Bacc("TRN2", target_bir_lowering=False, debug=False,
                       num_devices=N_CORES)
        build_program(nc)
        nc.compile()
        _CACHED = nc
    return _CACHED


def _prep_inputs(x, Wq, Wk, Wv, Wproj, bproj, W1, b1, W2, b2):
    bf = ml_dtypes.bfloat16
    W1t = np.ascontiguousarray(W1.astype(bf).reshape(CC, 128, F))
    W2t = np.ascontiguousarray(W2.astype(bf).reshape(FB, 128, C))
    # biases as [128, n] tiles: column i = bias block i
    b1r = np.ascontiguousarray(b1.astype(np.float32).reshape(FB, 128).T)
    b2r = np.ascontiguousarray(b2.astype(np.float32).reshape(CC, 128).T)
    bpr = np.ascontiguousarray(bproj.astype(np.float32).reshape(CC, 128).T)

    # wp[hh][t] rows: head 4t+hh (rows 0-63), head 4t+2+hh (rows 64-127)
    Wph = Wproj.reshape(H, HS, C)
    wp_s = np.empty((2, 4, 128, C), dtype=bf)
    for hh in range(2):
        for t4 in range(4):
            wp_s[hh, t4, 0:64] = Wph[4 * t4 + hh].astype(bf)
            wp_s[hh, t4, 64:128] = Wph[4 * t4 + 2 + hh].astype(bf)
    wp_s = np.ascontiguousarray(wp_s)

    f8 = ml_dtypes.float8_e4m3fn
    xT = [np.ascontiguousarray(x[b].T.astype(np.float32)) for b in range(B)]
    # fp8 DoubleRow layout: [4 cc-pairs, 128 partitions, 2 k-subtiles, T]
    xT_bf = np.ascontiguousarray(
        np.stack([xT[b].astype(f8).reshape(4, 2, 128, T).transpose(0, 2, 1, 3)
                  for b in range(B)]))

    in_maps = []
    for core in range(N_CORES):
        b, r = core // 4, core % 4
        cols = slice(128 * core, 128 * (core + 1))
        wqkv = np.empty((CC, 128, 384), dtype=f8)
        wqkv[:, :, 0:128] = (32 * Wq[:, cols]).astype(f8).reshape(CC, 128, 128)
        wqkv[:, :, 128:256] = (32 * Wk[:, cols]).astype(f8).reshape(CC, 128, 128)
        wqkv[:, :, 256:384] = (32 * Wv[:, cols]).astype(f8).reshape(CC, 128, 128)
        wqkv = np.ascontiguousarray(
            wqkv.reshape(4, 2, 128, 384).transpose(0, 2, 1, 3))
        xts = np.ascontiguousarray(
            xT[b][:, TS * r: TS * (r + 1)].reshape(CC, 128, TS))
        in_maps.append({
            "xt_bf": xT_bf, "xts_f": xts,
            "wqkv": wqkv, "wp": wp_s,
            "w1": W1t, "w2": W2t, "bp": bpr, "b1": b1r, "b2": b2r,
        })
    return in_maps


def kernel(x, Wq, Wk, Wv, Wproj, bproj, W1, b1, W2, b2, _trace=False):
    nc = _get_compiled()
    in_maps = _prep_inputs(np.asarray(x), np.asarray(Wq), np.asarray(Wk),
                           np.asarray(Wv), np.asarray(Wproj), np.asarray(bproj),
                           np.asarray(W1), np.asarray(b1), np.asarray(W2),
                           np.asarray(b2))
    res = run_bass_kernel_spmd(nc, in_maps, list(range(N_CORES)), trace=_trace)
    out = np.empty((B, T, C), dtype=np.float32)
    for c in range(N_CORES):
        b, r = c // 4, c % 4
        shard = res.results[c]["outT"].reshape(C, TS)
        out[b, TS * r: TS * (r + 1), :] = shard.T
    if _trace:
        kernel.last_exec_time_ns = res.exec_time_ns
    return out


# revision 23
# speedup vs baseline: 1.0534x; 1.0534x over previous
"""Fused transformer block (attention + MLP) on 8 trn2 NeuronCores.

Sharding: tensor-parallel over heads across ALL 8 cores — core c owns heads
{2c, 2c+1} for BOTH batches. Attention runs head-within-pair (hh) major;
each core's pre-projection attention outputs are exchanged with TWO 8-core
AllToAlls (one per hh, 512KB each) so the first fully overlaps the second
half's compute. After the exchange core r holds all 16 heads for its
(batch, T-shard) = (r//4, r%4) slice of 512 tokens and runs projection +
full FFN locally (sequence parallel). Output shards are gathered on host.

On-chip layout is "T-last" (feature dim on partitions, tokens on the free
axis) so no transposes are ever needed. V is built in natural [T, hs]
layout with a ones-column appended so the softmax denominator falls out of
the same matmul. Logits are tiny (|s| < ~1), so softmax needs no max
subtraction.
"""

import sys

for _p in ("/opt/trn_rl_repo",):
    if _p not in sys.path:
        sys.path.append(_p)

import numpy as np
import ml_dtypes

import concourse.bass as bass
import concourse.tile as tile
from concourse import bacc, mybir
from concourse.bass_utils import run_bass_kernel_spmd

BF16 = mybir.dt.bfloat16
FP8 = mybir.dt.float8e4
F32 = mybir.dt.float32
DR = mybir.MatmulPerfMode.DoubleRow
AF = mybir.ActivationFunctionType
ALU = mybir.AluOpType

N_CORES = 8
B, T, C = 2, 2048, 1024
H, HS = 16, 64
F = 4 * C
TS = T // 4
CC = C // 128
FB = F // 128
SCALE = float(C) ** -0.5


def build_program(nc: bass.Bass):
    xt_bf = nc.dram_tensor("xt_bf", [B, 4, 128, 2, T], FP8, kind="ExternalInput").ap()
    xts_f = nc.dram_tensor("xts_f", [CC, 128, TS], F32, kind="ExternalInput").ap()
    wqkv_d = nc.dram_tensor("wqkv", [4, 128, 2, 384], FP8, kind="ExternalInput").ap()
    wp_d = nc.dram_tensor("wp", [2, 4, 128, C], BF16, kind="ExternalInput").ap()
    w1_d = nc.dram_tensor("w1", [CC, 128, F], BF16, kind="ExternalInput").ap()
    w2_d = nc.dram_tensor("w2", [FB, 128, C], BF16, kind="ExternalInput").ap()
    bp_d = nc.dram_tensor("bp", [128, CC], F32, kind="ExternalInput").ap()
    b1_d = nc.dram_tensor("b1", [128, FB], F32, kind="ExternalInput").ap()
    b2_d = nc.dram_tensor("b2", [128, CC], F32, kind="ExternalInput").ap()
    out_d = nc.dram_tensor("outT", [CC, 128, TS], F32, kind="ExternalOutput").ap()

    with tile.TileContext(nc) as tc:
        _emit(nc, tc, xt_bf, xts_f, wqkv_d, wp_d, w1_d, w2_d,
              bp_d, b1_d, b2_d, out_d)


def _emit(nc, tc, xt_bf, xts_f, wqkv_d, wp_d, w1_d, w2_d,
          bp_d, b1_d, b2_d, out_d):
    from contextlib import ExitStack

    ctx = ExitStack()
    with ctx:
        st = ctx.enter_context(tc.tile_pool(name="static", bufs=1))
        w2p = ctx.enter_context(tc.tile_pool(name="w2s", bufs=3))
        outp = ctx.enter_context(tc.tile_pool(name="outp", bufs=4))
        rcp = ctx.enter_context(tc.tile_pool(name="rcp", bufs=2))
        dram0 = ctx.enter_context(tc.tile_pool(name="dram0", bufs=1, space="DRAM"))
        dram1 = ctx.enter_context(tc.tile_pool(name="dram1", bufs=1, space="DRAM"))

        # attention-phase SBUF (freed for w1-late/hT/x1 afterwards)
        attn_sb_ctx = tc.tile_pool(name="attnsb", bufs=1)
        asb = attn_sb_ctx.__enter__()
        expp_ctx = tc.tile_pool(name="expp", bufs=12)
        expp = expp_ctx.__enter__()

        psc_ctx = tc.tile_pool(name="psc", bufs=3, space="PSUM")
        psc = psc_ctx.__enter__()
        ps_ctx = tc.tile_pool(name="ps", bufs=2, space="PSUM")
        ps = ps_ctx.__enter__()

        # ---- weight loads first on gpsimd (QKV matmuls gate on these)
        wqkv_sb = []
        for cp in range(4):
            t_ = asb.tile([128, 2, 384], FP8, tag=f"wqkv{cp}", name=f"wqkv_sb{cp}")
            nc.gpsimd.dma_start(t_[:], wqkv_d[cp])
            wqkv_sb.append(t_)
        wk_sb = [t[:, :, 128:256] for t in wqkv_sb]
        wq_sb = [t[:, :, 0:128] for t in wqkv_sb]
        wv_sb = [t[:, :, 256:384] for t in wqkv_sb]
        bp_all = st.tile([128, CC], F32, tag="bp", name="bp_all")
        nc.gpsimd.dma_start(bp_all[:], bp_d)
        b1_all = st.tile([128, FB], F32, tag="b1", name="b1_all")
        nc.gpsimd.dma_start(b1_all[:], b1_d)
        b2_all = st.tile([128, CC], F32, tag="b2", name="b2_all")
        nc.gpsimd.dma_start(b2_all[:], b2_d)

        # ---- x loads: batch 0 chunk-major first, batch 1 after
        xt_sb = [[asb.tile([128, 2, T], FP8, tag=f"xt{b}{cp}", name=f"xt_sb{b}{cp}")
                  for cp in range(4)] for b in range(B)]
        for b in range(B):
            for q4 in range(4):
                for cp in range(4):
                    nc.sync.dma_start(xt_sb[b][cp][:, :, q4 * 512:(q4 + 1) * 512],
                                      xt_bf[b, cp][:, :, q4 * 512:(q4 + 1) * 512])
        wp_sb = [[None] * 4 for _ in range(2)]
        for hh in range(2):
            for t4 in range(4):
                t_ = st.tile([128, C], BF16, tag=f"wp{hh}_{t4}",
                             name=f"wp_sb{hh}_{t4}")
                nc.sync.dma_start(t_[:], wp_d[hh, t4])
                wp_sb[hh][t4] = t_
        xs_sb = []
        for cc in range(CC):
            xs = st.tile([128, TS], F32, tag=f"xs{cc}", name=f"xs{cc}")
            nc.sync.dma_start(xs[:], xts_f[cc])
            xs_sb.append(xs)
        mask_big = st.tile([128, 896], BF16, tag="mask", name="mask_big")
        nc.gpsimd.memset(mask_big[:], 1.0)
        nc.gpsimd.affine_select(mask_big[:], mask_big[:], pattern=[[1, 896]],
                                compare_op=ALU.is_ge, fill=0.0, base=-384,
                                channel_multiplier=-1)

        qT = [asb.tile([128, T], BF16, tag=f"qT{b}", name=f"qT_sb{b}")
              for b in range(B)]
        kT = [asb.tile([128, T], BF16, tag=f"kT{b}", name=f"kT_sb{b}")
              for b in range(B)]
        attnT = [asb.tile([128, T], BF16, tag=f"attnT{b}", name=f"attnT_sb{b}")
                 for b in range(B)]
        v_sb = [[None] * (T // 128) for _ in range(B)]
        at_sb = [[None] * 4 for _ in range(2)]
        exq = {}  # (b, qt, hh) -> list of exp tiles

        a2a_in = [dram0.tile([512, TS], BF16, name="a2a_in0"),
                  dram1.tile([512, TS], BF16, name="a2a_in1")]
        a2a_out = [dram0.tile([512, TS], BF16, name="a2a_out0"),
                   dram1.tile([512, TS], BF16, name="a2a_out1")]

        # w1 cc 0-3 first halves: loaded early (during attention)
        w1_sb = [[None, None] for _ in range(CC)]
        for cc in range(4):
            w1_sb[cc][0] = st.tile([128, 2048], BF16, tag=f"w1e{cc}",
                                   name=f"w1_sb{cc}_0")

        def qk(b, half):
            for nm, w_sb, dst in (("k", wk_sb, kT), ("q", wq_sb, qT)):
                acc = psc.tile([128, 1024], F32, tag="psc",
                               name=f"ps_{nm}{b}{half}")
                for t2 in range(2):
                    tt = 2 * half + t2
                    for cp in range(4):
                        nc.tensor.matmul(
                            acc[:, t2 * 512:(t2 + 1) * 512],
                            w_sb[cp][:],
                            xt_sb[b][cp][:, :, tt * 512:(tt + 1) * 512],
                            start=(cp == 0), stop=(cp == 3),
                            perf_mode=DR,
                        )
                nc.vector.tensor_copy(
                    dst[b][:, half * 1024:(half + 1) * 1024], acc[:])

        def v_tiles(b, tk0, tk1):
            for tk in range(tk0, tk1):
                vt = asb.tile([128, 2 * 65], BF16, tag=f"v{b}{tk}",
                              name=f"v_sb{b}{tk}")
                nc.gpsimd.memset(vt[:], 32.0)
                acc = ps.tile([128, 128], F32, tag="ps", name=f"ps_v{b}{tk}")
                for cp in range(4):
                    nc.tensor.matmul(
                        acc[:],
                        xt_sb[b][cp][:, :, tk * 128:(tk + 1) * 128],
                        wv_sb[cp][:],
                        start=(cp == 0), stop=(cp == 3),
                        perf_mode=DR,
                    )
                src = acc.rearrange("p (h d) -> p h d", h=2)
                dstv = vt.rearrange("p (h d) -> p h d", h=2, d=65)[:, :, 0:64]
                nc.vector.tensor_copy(dstv, src)
                v_sb[b][tk] = vt

        def sc(b, qt, hh):
            nkc = 4 * (qt + 1)
            p0 = 64 * hh
            for b2i in range(nkc // 2):
                sct = psc.tile([128, 1024], F32, tag="psc",
                               name=f"psc{b}{hh}{qt}_{b2i}")
                for j in range(2):
                    kc = 2 * b2i + j
                    nc.tensor.matmul(
                        sct[:, j * 512:(j + 1) * 512],
                        kT[b][p0:p0 + 64, kc * 128:(kc + 1) * 128],
                        qT[b][p0:p0 + 64, qt * 512:(qt + 1) * 512],
                        start=True, stop=True,
                    )
                ext = expp.tile([128, 1024], BF16, tag="expp",
                                name=f"ex{b}{hh}{qt}_{b2i}")
                nc.scalar.activation(ext[:], sct[:], AF.Exp, scale=SCALE / 1024.0)
                for j in range(2):
                    kc = 2 * b2i + j
                    if kc >= 4 * qt:
                        dd = kc * 128 - qt * 512
                        nc.vector.tensor_mul(
                            ext[:, j * 512:(j + 1) * 512],
                            ext[:, j * 512:(j + 1) * 512],
                            mask_big[:, 384 - dd:896 - dd],
                        )
                exq.setdefault((b, qt, hh), []).append(ext)

        def wv(b, qt, hh):
            nkc = 4 * (qt + 1)
            p0 = 64 * hh
            wv_acc = ps.tile([65, 512], F32, tag="ps", name=f"pwv{b}{hh}{qt}")
            for b2i in range(nkc // 2):
                ext = exq[(b, qt, hh)][b2i]
                for j in range(2):
                    kc = 2 * b2i + j
                    nc.tensor.matmul(
                        wv_acc[:],
                        v_sb[b][kc][:, hh * 65:hh * 65 + 65],
                        ext[:, j * 512:(j + 1) * 512],
                        start=(kc == 0), stop=(kc == nkc - 1),
                    )
            den = rcp.tile([1, 512], F32, tag="den", name=f"den{b}{hh}{qt}")
            nc.vector.tensor_copy(den[:], wv_acc[64:65, :])
            rc = rcp.tile([1, 512], F32, tag="rc", name=f"rc{b}{hh}{qt}")
            nc.vector.reciprocal_approx_fast(rc[:], den[:])
            rb = rcp.tile([64, 512], F32, tag="rb", name=f"rb{b}{hh}{qt}")
            nc.gpsimd.partition_broadcast(rb[:], rc[:])
            nc.vector.tensor_mul(
                attnT[b][p0:p0 + 64, qt * 512:(qt + 1) * 512],
                wv_acc[0:64, :], rb[:],
            )
            # ship this (b, q-tile, hh) slice to destination core 4b+qt
            j = 4 * b + qt
            nc.sync.dma_start(a2a_in[hh][j * 64:(j + 1) * 64, :],
                              attnT[b][p0:p0 + 64, qt * 512:(qt + 1) * 512])

        # ---- attention, hh-major ----
        qk(0, 0)
        v_tiles(0, 0, 4)
        sc(0, 0, 0)
        wv(0, 0, 0)
        qk(0, 1)
        v_tiles(0, 4, 8)
        sc(0, 1, 0)
        wv(0, 1, 0)
        qk(1, 0)
        v_tiles(0, 8, 12)
        sc(0, 2, 0)
        wv(0, 2, 0)
        qk(1, 1)
        v_tiles(0, 12, 16)
        sc(0, 3, 0)
        wv(0, 3, 0)
        v_tiles(1, 0, 4)
        v_tiles(1, 4, 8)
        sc(1, 0, 0)
        wv(1, 0, 0)
        v_tiles(1, 8, 12)
        sc(1, 1, 0)
        wv(1, 1, 0)
        v_tiles(1, 12, 16)
        # w1 cc0-3 first-half loads: transfers overlap late attention
        for cc in range(4):
            nc.gpsimd.dma_start(w1_sb[cc][0][:], w1_d[cc][:, 0:2048])
        sc(1, 2, 0)
        wv(1, 2, 0)
        sc(1, 3, 0)
        wv(1, 3, 0)

        nc.gpsimd.collective_compute(
            "AllToAll", ALU.bypass,
            replica_groups=[[0, 1, 2, 3, 4, 5, 6, 7]],
            ins=[a2a_in[0].opt()], outs=[a2a_out[0].opt()],
        )
        # emitted before collective #2 so they only gate on #1; the hh=1
        # a2a stores queue behind them, which is harmless (needed late)
        for t4 in range(4):
            t_ = st.tile([128, TS], BF16, tag=f"at0_{t4}", name=f"at0_{t4}")
            nc.sync.dma_start(t_[:], a2a_out[0][t4 * 128:(t4 + 1) * 128, :])
            at_sb[0][t4] = t_

        for b in range(B):
            for qt in range(4):
                sc(b, qt, 1)
                wv(b, qt, 1)

        nc.gpsimd.collective_compute(
            "AllToAll", ALU.bypass,
            replica_groups=[[0, 1, 2, 3, 4, 5, 6, 7]],
            ins=[a2a_in[1].opt()], outs=[a2a_out[1].opt()],
        )

        # attention pools done; free SBUF + PSUM for proj/FFN
        ps_ctx.__exit__(None, None, None)
        psc_ctx.__exit__(None, None, None)
        expp_ctx.__exit__(None, None, None)
        attn_sb_ctx.__exit__(None, None, None)

        late_ctx = tc.tile_pool(name="late", bufs=1)
        late = late_ctx.__enter__()

        # these wait on AllToAll #2; only later w1/w2 loads queue behind them
        for t4 in range(4):
            t_ = st.tile([128, TS], BF16, tag=f"at1_{t4}", name=f"at1_{t4}")
            nc.sync.dma_start(t_[:], a2a_out[1][t4 * 128:(t4 + 1) * 128, :])
            at_sb[1][t4] = t_
        # w1 cc4-7 first halves: issued after at1 so their 2MB of DMA
        # traffic stays out of AllToAll #2's transfer window
        for cc in range(4, CC):
            w1_sb[cc][0] = late.tile([128, 2048], BF16, tag=f"w1l{cc}",
                                     name=f"w1_sb{cc}_0")
            nc.sync.dma_start(w1_sb[cc][0][:], w1_d[cc][:, 0:2048])

        # ---- proj on the destination core: full Wproj @ attn T-shard ----
        # hh=0 half overlaps the second AllToAll.
        projps_ctx = tc.tile_pool(name="projps", bufs=8, space="PSUM")
        projps = projps_ctx.__enter__()
        pacc = [projps.tile([128, TS], F32, tag=f"pp{cb}", bufs=1,
                            name=f"ps_pj{cb}") for cb in range(CC)]
        x1f = [None] * CC
        x1b = [None] * CC
        for hh in range(2):
            for t4 in range(4):
                for cb in range(CC):
                    nc.tensor.matmul(
                        pacc[cb][:], wp_sb[hh][t4][:, cb * 128:(cb + 1) * 128],
                        at_sb[hh][t4][:],
                        start=(hh == 0 and t4 == 0), stop=(hh == 1 and t4 == 3))
                    if hh == 1 and t4 == 3:
                        xf = late.tile([128, TS], F32, tag=f"x1f{cb}",
                                       name=f"x1f{cb}")
                        nc.vector.scalar_tensor_tensor(
                            xf[:], pacc[cb][:], bp_all[:, cb:cb+1], xs_sb[cb][:],
                            ALU.add, ALU.add)
                        x1f[cb] = xf
                        xb = late.tile([128, TS], BF16, tag=f"x1b{cb}",
                                       name=f"x1b{cb}")
                        nc.vector.tensor_copy(xb[:], xf[:])
                        x1b[cb] = xb
        projps_ctx.__exit__(None, None, None)

        for cc in range(CC):
            w1_sb[cc][1] = late.tile([128, 2048], BF16, tag=f"w1m{cc}",
                                     name=f"w1_sb{cc}_1")
            nc.sync.dma_start(w1_sb[cc][1][:], w1_d[cc][:, 2048:4096])


        # ---- FFN1 with FFN2 group A (cb 0-3) interleaved ----
        fps_ctx = tc.tile_pool(name="fps", bufs=2, space="PSUM")
        fps = fps_ctx.__enter__()
        pf2a_ctx = tc.tile_pool(name="pf2a", bufs=6, space="PSUM")
        pf2a = pf2a_ctx.__enter__()
        accA = [pf2a.tile([128, TS], F32, tag=f"pfa{cb}", bufs=1,
                          name=f"ps_oa{cb}") for cb in range(6)]
        hT = [late.tile([128, TS], BF16, tag=f"hT{fb}", name=f"hT{fb}")
              for fb in range(FB)]
        for fb in range(FB):
            w1h, fo = fb // 16, fb % 16
            acc = fps.tile([128, TS], F32, tag="fps", name=f"ps_h{fb}")
            for cc in range(CC):
                nc.tensor.matmul(
                    acc[:],
                    w1_sb[cc][w1h][:, fo * 128:(fo + 1) * 128],
                    x1b[cc][:],
                    start=(cc == 0), stop=(cc == CC - 1))
            nc.scalar.activation(hT[fb][:], acc[:], AF.Relu, bias=b1_all[:, fb:fb+1])
            wt = w2p.tile([128, 768], BF16, tag="w2s", name=f"w2ta{fb}")
            nc.sync.dma_start(wt[:], w2_d[fb][:, 0:768])
            for cb in range(6):
                nc.tensor.matmul(
                    accA[cb][:], wt[:, cb * 128:(cb + 1) * 128], hT[fb][:],
                    start=(fb == 0), stop=(fb == FB - 1))
        for cb in range(6):
            ot = outp.tile([128, TS], F32, tag="outp", name=f"ot{cb}")
            nc.vector.scalar_tensor_tensor(ot[:], accA[cb][:], b2_all[:, cb:cb+1],
                                           x1f[cb][:], ALU.add, ALU.add)
            nc.sync.dma_start(out_d[cb], ot[:])
        pf2a_ctx.__exit__(None, None, None)

        # ---- FFN2 group B ----
        pf2_ctx = tc.tile_pool(name="pf2", bufs=2, space="PSUM")
        pf2 = pf2_ctx.__enter__()
        accB = [pf2.tile([128, TS], F32, tag=f"pfb{cb}", bufs=1,
                         name=f"ps_ob{cb}") for cb in range(2)]
        for fc in range(FB):
            wt = w2p.tile([128, 256], BF16, tag="w2s", name=f"w2tb{fc}")
            nc.sync.dma_start(wt[:], w2_d[fc][:, 768:1024])
            for cb in range(2):
                nc.tensor.matmul(
                    accB[cb][:], wt[:, cb * 128:(cb + 1) * 128], hT[fc][:],
                    start=(fc == 0), stop=(fc == FB - 1))
        for cb2 in range(2):
            cb = cb2 + 6
            ot = outp.tile([128, TS], F32, tag="outp", name=f"ot{cb}")
            nc.vector.scalar_tensor_tensor(ot[:], accB[cb2][:], b2_all[:, cb:cb+1],
                                           x1f[cb][:], ALU.add, ALU.add)
            nc.sync.dma_start(out_d[cb], ot[:])
        pf2_ctx.__exit__(None, None, None)
        fps_ctx.__exit__(None, None, None)
        late_ctx.__exit__(None, None, None)


_CACHED = None


def _get_compiled():
    global _CACHED
    if _CACHED is None:
        nc = bacc.Bacc("TRN2", target_bir_lowering=False, debug=False,
                       num_devices=N_CORES)
        build_program(nc)
        nc.compile()
        _CACHED = nc
    return _CACHED


def _prep_inputs(x, Wq, Wk, Wv, Wproj, bproj, W1, b1, W2, b2):
    bf = ml_dtypes.bfloat16
    W1t = np.ascontiguousarray(W1.astype(bf).reshape(CC, 128, F))
    W2t = np.ascontiguousarray(W2.astype(bf).reshape(FB, 128, C))
    # biases as [128, n] tiles: column i = bias block i
    b1r = np.ascontiguousarray(b1.astype(np.float32).reshape(FB, 128).T)
    b2r = np.ascontiguousarray(b2.astype(np.float32).reshape(CC, 128).T)
    bpr = np.ascontiguousarray(bproj.astype(np.float32).reshape(CC, 128).T)

    # wp[hh][t] rows: head 4t+hh (rows 0-63), head 4t+2+hh (rows 64-127)
    Wph = Wproj.reshape(H, HS, C)
    wp_s = np.empty((2, 4, 128, C), dtype=bf)
    for hh in range(2):
        for t4 in range(4):
            wp_s[hh, t4, 0:64] = Wph[4 * t4 + hh].astype(bf)
            wp_s[hh, t4, 64:128] = Wph[4 * t4 + 2 + hh].astype(bf)
    wp_s = np.ascontiguousarray(wp_s)

    f8 = ml_dtypes.float8_e4m3fn
    xT = [np.ascontiguousarray(x[b].T.astype(np.float32)) for b in range(B)]
    # fp8 DoubleRow layout: [4 cc-pairs, 128 partitions, 2 k-subtiles, T]
    xT_bf = np.ascontiguousarray(
        np.stack([xT[b].astype(f8).reshape(4, 2, 128, T).transpose(0, 2, 1, 3)
                  for b in range(B)]))

    in_maps = []
    for core in range(N_CORES):
        b, r = core // 4, core % 4
        cols = slice(128 * core, 128 * (core + 1))
        wqkv = np.empty((CC, 128, 384), dtype=f8)
        wqkv[:, :, 0:128] = (32 * Wq[:, cols]).astype(f8).reshape(CC, 128, 128)
        wqkv[:, :, 128:256] = (32 * Wk[:, cols]).astype(f8).reshape(CC, 128, 128)
        wqkv[:, :, 256:384] = (32 * Wv[:, cols]).astype(f8).reshape(CC, 128, 128)
        wqkv = np.ascontiguousarray(
            wqkv.reshape(4, 2, 128, 384).transpose(0, 2, 1, 3))
        xts = np.ascontiguousarray(
            xT[b][:, TS * r: TS * (r + 1)].reshape(CC, 128, TS))
        in_maps.append({
            "xt_bf": xT_bf, "xts_f": xts,
            "wqkv": wqkv, "wp": wp_s,
            "w1": W1t, "w2": W2t, "bp": bpr, "b1": b1r, "b2": b2r,
        })
    return in_maps


def kernel(x, Wq, Wk, Wv, Wproj, bproj, W1, b1, W2, b2, _trace=False):
    nc = _get_compiled()
    in_maps = _prep_inputs(np.asarray(x), np.asarray(Wq), np.asarray(Wk),
                           np.asarray(Wv), np.asarray(Wproj), np.asarray(bproj),
                           np.asarray(W1), np.asarray(b1), np.asarray(W2),
                           np.asarray(b2))
    res = run_bass_kernel_spmd(nc, in_maps, list(range(N_CORES)), trace=_trace)
    out = np.empty((B, T, C), dtype=np.float32)
    for c in range(N_CORES):
        b, r = c // 4, c % 4
        shard = res.results[c]["outT"].reshape(C, TS)
        out[b, TS * r: TS * (r + 1), :] = shard.T
    if _trace:
        kernel.last_exec_time_ns = res.exec_time_ns
    return out
